# revision 15
# baseline (speedup 1.0000x reference)
"""DiffSAGE GNN layer on 8 Trainium2 NeuronCores.

Math (per reference):
    msg      = x[src] - x[dst]                      # per edge
    agg      = segment_mean(msg, dst, N)            # zeros where cnt==0
    out      = agg @ Wl.T + bl + x @ Wr.T

Identity used by the kernel:
    sum_{e: dst=i} (x[src_e] - x[i]) = gsum[i] - cnt[i]*x[i]
    agg[i] = gsum[i] * r[i] - x[i] * s[i]
      where r = 1/max(cnt,1), s = cnt*r  (s is 0 or 1)

Distribution: destination-node sharding across the 8 cores (core c owns node
rows [c*N/8, (c+1)*N/8)).  Each core gathers x[src] rows for its own edges
from a replicated table, so no collectives are needed.

Per-core device algorithm (single compiled SPMD program, data-uniform):
  - nodes are cut into aligned 64-node windows; every window gets a fixed
    budget of CAPA chunks of "A" edges (src < 32767) and CAPB chunks of "B"
    edges (src >= 32767) -- dma_gather indices are int16, so x is staged as
    two bf16 half-tables, each with a zero row at index 0 used for padding.
  - per window group, one dma_gather per half-table pulls all slot rows into
    SBUF with row i landing at [partition i%128, slot i//128].
  - per 128-edge chunk: one-hot[e, n] = (dst_rel[e] == iota[n]) on DVE, then
    PE accumulates  psum_agg[feat, node] += msg_chunk.T @ one-hot  and
    psum_cnt[0, node] += ones_col.T @ one-hot  (padded slots carry
    dst_rel=999 -> all-zero one-hot column -> no contribution).
  - window epilogue: r/s from cnt, rank-1 matmul broadcasts them across
    partitions, agg_T = psum_agg*r - xT_slab*s, two matmuls apply Wl/Wr,
    bias add, PE transpose back to [node, feat], contiguous store.
"""

import sys

import numpy as np

try:
    import concourse.bass as bass
except Exception:  # pragma: no cover - harness path setup
    for p in (
        "/root/.axon_site",
        "/root/.axon_site/_ro/trn_rl_repo",
        "/root/.axon_site/_ro/pypackages",
        "/opt/trn_rl_repo",
    ):
        if p not in sys.path:
            sys.path.append(p)
    import concourse.bass as bass

from contextlib import ExitStack

import ml_dtypes

import concourse.mybir as mybir
import concourse.tile as tile
from concourse import bacc, bass_utils
from concourse.masks import make_identity

F32 = mybir.dt.float32
BF16 = mybir.dt.bfloat16
I16 = mybir.dt.int16

D = 128          # feature dim (in and out)
WN = 64          # nodes per window
CHUNK = 128      # edges per matmul chunk (contraction dim)
SPLIT = 32767    # x rows < SPLIT go to table A, rest to table B
GROUP = 2        # windows whose gathers are batched into one dma_gather


class Cfg:
    def __init__(self, nta, ntb, wins, capa, capb, n_cores, group=GROUP):
        self.NTA = nta          # rows in table A (incl zero row)
        self.NTB = ntb          # rows in table B (incl zero row)
        self.WINS = wins        # 64-node windows per core
        self.CAPA = capa        # A-chunks per window
        self.CAPB = capb        # B-chunks per window
        self.NSLAB = wins * WN  # padded nodes per core
        self.N_CORES = n_cores
        self.G = group
        assert wins % group == 0


def build_nc(cfg: Cfg, repeat: int = 1) -> bass.Bass:
    nc = bacc.Bacc("TRN2")
    WINS, CAPA, CAPB, NSLAB, G = cfg.WINS, cfg.CAPA, cfg.CAPB, cfg.NSLAB, cfg.G
    CAP = CAPA + CAPB
    NCALLS = WINS // G
    NIA = G * CAPA * CHUNK   # idxs per A-call
    NIB = G * CAPB * CHUNK

    taba = nc.dram_tensor("taba", [cfg.NTA, D], BF16, kind="ExternalInput")
    tabb = nc.dram_tensor("tabb", [cfg.NTB, D], BF16, kind="ExternalInput")
    idxa = nc.dram_tensor("idxa", [NCALLS, CHUNK, NIA // 16], I16, kind="ExternalInput")
    idxb = nc.dram_tensor("idxb", [NCALLS, CHUNK, NIB // 16], I16, kind="ExternalInput")
    xts = nc.dram_tensor("xts", [D, NSLAB], F32, kind="ExternalInput")
    dstr = nc.dram_tensor("dstr", [WINS, CHUNK, CAP], F32, kind="ExternalInput")
    wlt = nc.dram_tensor("wlt", [D, D], F32, kind="ExternalInput")
    wrt = nc.dram_tensor("wrt", [D, D], F32, kind="ExternalInput")
    blb = nc.dram_tensor("blb", [D, 1], F32, kind="ExternalInput")
    outh = nc.dram_tensor("out", [NSLAB, D], F32, kind="ExternalOutput")

    with ExitStack() as ctx:
        tc = ctx.enter_context(tile.TileContext(nc))
        singles = ctx.enter_context(tc.tile_pool(name="singles", bufs=1))
        stage_p = ctx.enter_context(tc.tile_pool(name="stage", bufs=2))
        io_p = ctx.enter_context(tc.tile_pool(name="io", bufs=3))
        oh_p = ctx.enter_context(tc.tile_pool(name="oh", bufs=4))
        wrk = ctx.enter_context(tc.tile_pool(name="wrk", bufs=3))
        pacc = ctx.enter_context(tc.tile_pool(name="pacc", bufs=2, space="PSUM"))
        pepi = ctx.enter_context(tc.tile_pool(name="pepi", bufs=2, space="PSUM"))

        # ---- one-time constants ----
        xt_sb = singles.tile([D, NSLAB], F32)
        nc.sync.dma_start(out=xt_sb[:], in_=xts[:])
        wlt_sb = singles.tile([D, D], F32)
        nc.sync.dma_start(out=wlt_sb[:], in_=wlt[:])
        wrt_sb = singles.tile([D, D], F32)
        nc.sync.dma_start(out=wrt_sb[:], in_=wrt[:])
        bl_sb = singles.tile([D, 1], F32)
        nc.sync.dma_start(out=bl_sb[:], in_=blb[:])
        ident = singles.tile([D, D], F32)
        make_identity(nc, ident[:])
        ones_row = singles.tile([1, D], F32)
        nc.vector.memset(ones_row[:], 1.0)
        ones_col = singles.tile([CHUNK, 1], BF16)
        nc.vector.memset(ones_col[:], 1.0)
        iota_i = singles.tile([CHUNK, WN], I32 := mybir.dt.int32)
        nc.gpsimd.iota(iota_i[:], pattern=[[1, WN]], channel_multiplier=0)
        iota_f = singles.tile([CHUNK, WN], F32)
        nc.vector.tensor_copy(out=iota_f[:], in_=iota_i[:])

        state = {}

        def window_body(w):
            g, wg = divmod(w, G)
            if wg == 0:
                ia_t = io_p.tile([CHUNK, NIA // 16], I16, tag="ia")
                nc.sync.dma_start(out=ia_t[:], in_=idxa[g])
                ib_t = io_p.tile([CHUNK, NIB // 16], I16, tag="ib")
                nc.sync.dma_start(out=ib_t[:], in_=idxb[g])
                stga = stage_p.tile([CHUNK, G * CAPA, D], BF16, tag="stga")
                nc.gpsimd.dma_gather(
                    stga[:], taba[:], ia_t[:], NIA, NIA, D, single_packet=False
                )
                state["stga"] = stga
                stgb = stage_p.tile([CHUNK, G * CAPB, D], BF16, tag="stgb")
                nc.gpsimd.dma_gather(
                    stgb[:], tabb[:], ib_t[:], NIB, NIB, D, single_packet=False
                )
                state["stgb"] = stgb

            dst_t = io_p.tile([CHUNK, CAP], F32, tag="dst")
            nc.sync.dma_start(out=dst_t[:], in_=dstr[w])

            agg_ps = pacc.tile([D, WN], F32, space="PSUM", tag="agg")
            epi_ps = pepi.tile([D, 384], F32, space="PSUM", tag="epi")
            for c in range(CAP):
                if c < CAPA:
                    msg = state["stga"][:, wg * CAPA + c, :]
                else:
                    msg = state["stgb"][:, wg * CAPB + (c - CAPA), :]
                oh = oh_p.tile([CHUNK, WN], BF16, tag="oh")
                nc.vector.tensor_tensor(
                    out=oh[:],
                    in0=dst_t[:, c : c + 1].to_broadcast([CHUNK, WN]),
                    in1=iota_f[:],
                    op=mybir.AluOpType.is_equal,
                )
                nc.tensor.matmul(
                    agg_ps[:],
                    lhsT=msg,
                    rhs=oh[:],
                    start=(c == 0),
                    stop=(c == CAP - 1),
                )
                nc.tensor.matmul(
                    epi_ps[0:1, 0:WN],
                    lhsT=ones_col[:],
                    rhs=oh[:],
                    start=(c == 0),
                    stop=(c == CAP - 1),
                )

            # ---- window epilogue ----
            # tmp: [cnt_max | r | s]
            tmp = wrk.tile([1, 3 * WN], F32, tag="tmp")
            nc.vector.tensor_scalar_max(tmp[:, 0:WN], epi_ps[0:1, 0:WN], 1.0)
            nc.vector.reciprocal(tmp[:, WN : 2 * WN], tmp[:, 0:WN])
            nc.vector.tensor_tensor(
                out=tmp[:, 2 * WN : 3 * WN],
                in0=epi_ps[0:1, 0:WN],
                in1=tmp[:, WN : 2 * WN],
                op=mybir.AluOpType.mult,
            )
            # broadcast [r | s] to all 128 partitions: epi[:, 64:192]
            nc.tensor.matmul(
                epi_ps[:, WN : WN + 2 * WN],
                lhsT=ones_row[:],
                rhs=tmp[:, WN : 3 * WN],
                start=True,
                stop=True,
            )
            rs_sb = wrk.tile([D, 2 * WN], F32, tag="rs")
            nc.vector.tensor_copy(out=rs_sb[:], in_=epi_ps[:, WN : 3 * WN])
            xw = xt_sb[:, w * WN : (w + 1) * WN]

            agg_sb = wrk.tile([D, WN], F32, tag="aggs")
            xs_sb = wrk.tile([D, WN], F32, tag="xs")
            nc.vector.tensor_tensor(
                out=agg_sb[:], in0=agg_ps[:], in1=rs_sb[:, 0:WN], op=mybir.AluOpType.mult
            )
            nc.vector.tensor_tensor(
                out=xs_sb[:], in0=xw, in1=rs_sb[:, WN : 2 * WN], op=mybir.AluOpType.mult
            )
            agg2_sb = wrk.tile([D, WN], F32, tag="agg2")
            nc.vector.tensor_tensor(
                out=agg2_sb[:],
                in0=agg_sb[:],
                in1=xs_sb[:],
                op=mybir.AluOpType.subtract,
            )

            # linear: outT = Wl @ agg + Wr @ x  into epi[:, 192:256]
            lin = epi_ps[:, 3 * WN : 4 * WN]
            nc.tensor.matmul(lin, lhsT=wlt_sb[:], rhs=agg2_sb[:], start=True, stop=False)
            nc.tensor.matmul(lin, lhsT=wrt_sb[:], rhs=xw, start=False, stop=True)

            outt = wrk.tile([D, WN], F32, tag="outt")
            nc.vector.tensor_scalar_add(outt[:], lin, bl_sb[:, 0:1])

            # transpose back to [node, feat] and store
            tr = epi_ps[0:WN, 4 * WN : 4 * WN + D]
            nc.tensor.transpose(tr, outt[:], ident[:])
            outr = wrk.tile([WN, D], F32, tag="outr")
            nc.vector.tensor_copy(out=outr[:], in_=tr)
            nc.sync.dma_start(out=outh[w * WN : (w + 1) * WN, :], in_=outr[:])

        if repeat > 1:
            rep_start = nc.snap(0)
            rep_end = nc.snap(repeat)
            with tc.For_i(rep_start, rep_end, 1, name="rep") as _rep_i:
                for w in range(WINS):
                    window_body(w)
        else:
            for w in range(WINS):
                window_body(w)

    nc.compile()
    return nc


def wrap_idx(idx):
    """[n] -> [128, n/16] int16 (16-partition wrap, replicated 8x)."""
    n = len(idx)
    w = idx.reshape(n // 16, 16)
    return np.ascontiguousarray(np.tile(w.T, (8, 1)))


def pack_side(gidx, win, wins, cap, order_key):
    """Pack one side's edges (gather indices `gidx`, window ids `win`) into
    per-window slot arrays.  Returns (slots [wins, cap*128] int16 gather idx,
    flat positions of real edges, their order)."""
    slots = cap * CHUNK
    counts = np.bincount(win, minlength=wins)
    assert counts.max() <= slots, (counts.max(), slots)
    order = np.argsort(win, kind="stable")
    starts = np.zeros(wins, dtype=np.int64)
    starts[1:] = np.cumsum(counts)[:-1]
    rank = np.arange(len(gidx)) - starts[win[order]]
    flat_pos = win[order] * slots + rank
    arr = np.zeros(wins * slots, dtype=np.int16)  # pad -> zero row
    arr[flat_pos] = gidx[order].astype(np.int16)
    return arr, flat_pos, order


def neg_tail_pads(arr_calls, counts_per_call_region):
    """Mark trailing pad slots of each gather call as -1 (skipped by HW)."""
    # arr_calls: [ncalls, nidx]; for each call, walk back while pad (0).
    for a in arr_calls:
        nz = np.nonzero(a)[0]
        end = (nz[-1] + 1) if len(nz) else 0
        a[end:] = -1
    return arr_calls


def run_graph(x, edge_index, Wl, bl, Wr, n_cores=8, group=GROUP, trace=False,
              min_capa=1, min_capb=1, repeat=1):
    """Full pipeline: host prep -> one SPMD compile -> run -> unshard."""
    x = np.asarray(x, dtype=np.float32)
    n, d = x.shape
    assert d == D
    src = np.asarray(edge_index[0], dtype=np.int64)
    dst = np.asarray(edge_index[1], dtype=np.int64)
    assert n % n_cores == 0
    npc = n // n_cores
    wins = -(-npc // WN)
    while wins % group:
        wins += 1
    nslab = wins * WN

    core_of = dst // npc
    ldst_all = dst - core_of * npc
    win_all = core_of * wins + ldst_all // WN
    is_a = src < SPLIT

    # capacities from global max window occupancy (uniform across cores)
    ca = np.bincount(win_all[is_a], minlength=n_cores * wins)
    cb = np.bincount(win_all[~is_a], minlength=n_cores * wins)
    capa = max(int(min_capa), int(-(-ca.max() // CHUNK)))
    capb = max(int(min_capb), int(-(-cb.max() // CHUNK)))
    na = min(n, SPLIT)
    ntb = max(n - SPLIT, 1) + 1
    cfg = Cfg(nta=na + 1, ntb=ntb, wins=wins, capa=capa, capb=capb,
              n_cores=n_cores, group=group)

    # shared tables (row 0 = zeros for padding)
    taba = np.zeros((na + 1, D), dtype=ml_dtypes.bfloat16)
    taba[1 : na + 1] = x[:na].astype(ml_dtypes.bfloat16)
    tabb = np.zeros((ntb, D), dtype=ml_dtypes.bfloat16)
    if n > SPLIT:
        tabb[1 : n - SPLIT + 1] = x[SPLIT:].astype(ml_dtypes.bfloat16)
    wlt = np.ascontiguousarray(np.asarray(Wl, np.float32).T)
    wrt = np.ascontiguousarray(np.asarray(Wr, np.float32).T)
    blb = np.ascontiguousarray(np.asarray(bl, np.float32).reshape(D, 1))

    ncalls = wins // group
    nia, nib = group * capa * CHUNK, group * capb * CHUNK
    in_maps = []
    for c in range(n_cores):
        m = core_of == c
        ms, mwin, ma = src[m], (ldst_all[m] // WN), is_a[m]
        mrel = (ldst_all[m] % WN).astype(np.float32)

        sa, posa, orda = pack_side(ms[ma] + 1, mwin[ma], wins, capa, None)
        sb, posb, ordb = pack_side(ms[~ma] - (SPLIT - 1), mwin[~ma], wins, capb, None)

        # combined dst_rel slots: [wins, (capa+capb)*128], pad 999
        dstr = np.full((wins, (capa + capb) * CHUNK), 999.0, dtype=np.float32)
        wa, ra = np.divmod(posa, capa * CHUNK)
        dstr[wa, ra] = mrel[ma][orda]
        wb, rb = np.divmod(posb, capb * CHUNK)
        dstr[wb, capa * CHUNK + rb] = mrel[~ma][ordb]

        # gather call arrays: [ncalls, group*cap*128]; pads gather row 0
        ia_w = np.stack([wrap_idx(a) for a in sa.reshape(ncalls, nia)])
        ib_w = np.stack([wrap_idx(b) for b in sb.reshape(ncalls, nib)])

        # dst tile layout: [wins, 128, cap] with slot c*128+p -> [p, c]
        cap = capa + capb
        dstr_t = np.ascontiguousarray(
            dstr.reshape(wins, cap, CHUNK).transpose(0, 2, 1)
        )

        xs = np.zeros((D, nslab), dtype=np.float32)
        xs[:, :npc] = x[c * npc : (c + 1) * npc].T

        in_maps.append(
            {
                "taba": taba,
                "tabb": tabb,
                "idxa": ia_w,
                "idxb": ib_w,
                "xts": xs,
                "dstr": dstr_t,
                "wlt": wlt,
                "wrt": wrt,
                "blb": blb,
            }
        )

    nc = build_nc(cfg, repeat=repeat)
    res = bass_utils.run_bass_kernel_spmd(
        nc, in_maps, core_ids=list(range(n_cores)), trace=trace
    )
    out = np.concatenate(
        [res.results[c]["out"][:npc] for c in range(n_cores)], axis=0
    )
    return np.ascontiguousarray(out, dtype=np.float32), res


class Runner:
    """Jit the compiled Bass program once; support repeated timed runs.

    Mirrors bass2jax.run_bass_via_pjrt's multi-core path, but keeps the
    jitted callable and pre-placed device inputs so subsequent calls measure
    device execution without retrace/recompile or H2D of the big tensors.
    """

    def __init__(self, nc, in_maps, n_cores):
        import jax
        import jax.numpy as jnp
        from jax.sharding import Mesh, NamedSharding, PartitionSpec
        from jax.experimental.shard_map import shard_map

        from concourse import bass2jax as B2J
        from concourse import mybir as _mb

        B2J.install_neuronx_cc_hook()
        self.jax = jax
        partition_name = (
            nc.partition_id_tensor.name if nc.partition_id_tensor else None
        )
        in_names, out_names, out_avals, zero_outs = [], [], [], []
        for alloc in nc.m.functions[0].allocations:
            if not isinstance(alloc, _mb.MemoryLocationSet):
                continue
            name = alloc.memorylocations[0].name
            if alloc.kind == "ExternalInput":
                if name != partition_name:
                    in_names.append(name)
            elif alloc.kind == "ExternalOutput":
                shape = tuple(alloc.tensor_shape)
                dtype = _mb.dt.np(alloc.dtype)
                out_names.append(name)
                out_avals.append(jax.core.ShapedArray(shape, dtype))
                zero_outs.append(np.zeros(shape, dtype))
        n_params = len(in_names)
        all_in_names = list(in_names) + out_names
        if partition_name is not None:
            all_in_names.append(partition_name)
        donate = tuple(range(n_params, n_params + len(out_avals)))

        def _body(*args):
            operands = list(args)
            if partition_name is not None:
                operands.append(B2J.partition_id_tensor())
            outs = B2J._bass_exec_p.bind(
                *operands,
                out_avals=tuple(out_avals),
                in_names=tuple(all_in_names),
                out_names=tuple(out_names),
                lowering_input_output_aliases=(),
                sim_require_finite=True,
                sim_require_nnan=True,
                nc=nc,
            )
            return tuple(outs)

        devices = jax.devices()[:n_cores]
        mesh = Mesh(np.asarray(devices), ("core",))
        self.mesh = mesh
        spec = PartitionSpec("core")
        in_specs = (spec,) * (n_params + len(out_avals))
        out_specs = (spec,) * len(out_names)
        self.fn = jax.jit(
            shard_map(
                _body, mesh=mesh, in_specs=in_specs, out_specs=out_specs,
                check_rep=False,
            ),
            donate_argnums=donate,
            keep_unused=True,
        )
        sharding = NamedSharding(mesh, spec)
        concat_in = [
            np.concatenate([np.asarray(m[name]) for m in in_maps], axis=0)
            for name in in_names
        ]
        self.dev_in = [jax.device_put(a, sharding) for a in concat_in]
        self.zero_outs = zero_outs
        self.sharding = sharding
        self.out_names = out_names
        self.out_avals = out_avals
        self.n_cores = n_cores

    def _zeros(self):
        return [
            self.jax.device_put(
                np.zeros((self.n_cores * z.shape[0], *z.shape[1:]), z.dtype),
                self.sharding,
            )
            for z in self.zero_outs
        ]

    def run(self):
        outs = self.fn(*self.dev_in, *self._zeros())
        self.jax.block_until_ready(outs)
        return outs

    def timed(self, iters=20):
        import time

        zero_sets = [self._zeros() for _ in range(iters)]
        self.jax.block_until_ready(zero_sets)
        outs = None
        times = []
        for i in range(iters):
            t0 = time.perf_counter()
            outs = self.fn(*self.dev_in, *zero_sets[i])
            self.jax.block_until_ready(outs)
            times.append(time.perf_counter() - t0)
        return outs, times

    def results(self, outs):
        res = []
        for c in range(self.n_cores):
            res.append(
                {
                    name: np.asarray(outs[i]).reshape(
                        self.n_cores, *self.out_avals[i].shape
                    )[c]
                    for i, name in enumerate(self.out_names)
                }
            )
        return res


def make_runner(x, edge_index, Wl, bl, Wr, n_cores=8, group=GROUP, repeat=1):
    """Build host data + compiled program + Runner (for timing loops)."""
    x = np.asarray(x, dtype=np.float32)
    saved = {}
    orig = bass_utils.run_bass_kernel_spmd

    def capture(nc, in_maps, core_ids, trace=False):
        saved["nc"], saved["in_maps"] = nc, in_maps
        raise _Captured()

    class _Captured(Exception):
        pass

    bass_utils.run_bass_kernel_spmd = capture
    try:
        run_graph(x, edge_index, Wl, bl, Wr, n_cores=n_cores, group=group,
                  repeat=repeat)
    except _Captured:
        pass
    finally:
        bass_utils.run_bass_kernel_spmd = orig
    return Runner(saved["nc"], saved["in_maps"], n_cores), saved


def kernel(**inputs) -> np.ndarray:
    out, _ = run_graph(
        inputs["x"],
        inputs["edge_index"],
        inputs["Wl"],
        inputs["bl"],
        inputs["Wr"],
        n_cores=8,
    )
    return out
